# revision 24
# baseline (speedup 1.0000x reference)
"""Trainium2 Bass kernel for attention-pooling:
    score  = tanh(X @ W)            [B,T,H]
    logits = score @ c              [B,T]
    attn   = softmax(logits, ax=1)  [B,T]
    ctx    = attn-weighted sum of X over T -> [B,H]
Returns (ctx, attn). Data-parallel over batch across 8 NeuronCores.

Per-core pipeline (4 samples, T=2048, H=1024):
  P0   gpsimd cast-DMA x f32 -> DRAM scratch bf16, tiled [hb][t][128]
  XT   one huge xbar DMA-transpose per (sample, h-block): [2048,128]->[128,2048]
  A    S^T = W^T X^T in bf16 (W stationary), tanh fused on ScalarE
  L    logits^T via N=1 matmuls -> [128t, 1] slices (softmax-friendly layout)
  SM   exp (no max-sub: |logits| <= sum|c| ~ 40), cross-partition sum via
       ones-matmul, reciprocal broadcast, attn = probs * recip
  P    ctx = attn^T X with attn[128,1] stationary tiles, X natural from scratch
"""

import numpy as np

B, T, H = 32, 2048, 1024
NCORES = 8
BC = B // NCORES            # samples per core
CHUNK = 512                 # t-rows per phase-A chunk
CH_PER_B = T // CHUNK       # 4 chunks per sample
HB = H // 128               # 8 h blocks
JJ = CHUNK // 128           # 4 128-t slices per chunk
SPS = T // 128              # 16 slices per sample

_cache = {}


def build():
    import concourse.bass as bass
    import concourse.tile as tile
    from concourse import bacc, mybir
    from contextlib import ExitStack

    f32 = mybir.dt.float32
    bf16 = mybir.dt.bfloat16
    AF = mybir.ActivationFunctionType
    AX = mybir.AxisListType

    nc = bacc.Bacc("TRN2", target_bir_lowering=False, debug=False)

    x = nc.declare_dram_parameter("x", [BC, T, H], f32, isOutput=False)
    w = nc.declare_dram_parameter("w", [H, H], f32, isOutput=False)
    c = nc.declare_dram_parameter("c", [H, 1], f32, isOutput=False)
    out_ctx = nc.declare_dram_parameter("out_ctx", [BC, H], f32, isOutput=True)
    out_attn = nc.declare_dram_parameter("out_attn", [BC, T], f32, isOutput=True)

    xf = x[:].rearrange("b t h -> (b t) h")  # [8192, 1024]

    with tile.TileContext(nc) as tc, ExitStack() as ctx:
        ep = ctx.enter_context
        const_pool = ep(tc.tile_pool(name="const", bufs=1))
        xt_pool = ep(tc.tile_pool(name="xt", bufs=24))
        st_pool = ep(tc.tile_pool(name="st", bufs=16))
        junk_pool = ep(tc.tile_pool(name="junk", bufs=2))
        sm_pool = ep(tc.tile_pool(name="sm", bufs=2))
        dram_pool = ep(tc.tile_pool(name="dram", bufs=4, space="DRAM"))
        mm_ps = ep(tc.tile_pool(name="mm_ps", bufs=3, space="PSUM"))
        lg_ps = ep(tc.tile_pool(name="lg_ps", bufs=1, space="PSUM"))
        abc_ps = ep(tc.tile_pool(name="abc_ps", bufs=2, space="PSUM"))
        tiny_ps = ep(tc.tile_pool(name="tiny_ps", bufs=2, space="PSUM"))

        # ---- constants / weights ----
        w_sb = const_pool.tile([128, HB, H], bf16, tag="w")
        nc.gpsimd.dma_start(
            out=w_sb[:], in_=w[:].rearrange("(hb p) h -> p hb h", p=128)
        )
        c_sb = const_pool.tile([128, HB], bf16, tag="c")
        nc.gpsimd.dma_start(
            out=c_sb[:], in_=c[:].rearrange("(a p) k -> p (a k)", p=128)
        )
        ones_col = const_pool.tile([128, 1], f32, tag="ones_col")
        nc.any.memset(ones_col[:], 1.0)
        ones_row = const_pool.tile([1, 128], f32, tag="ones_row")
        nc.any.memset(ones_row[:], 1.0)
        ones_row_b = const_pool.tile([1, 128], bf16, tag="ones_row_b")
        nc.any.memset(ones_row_b[:], 1.0)
        ident_f32 = const_pool.tile([128, 128], f32, tag="ident")
        from concourse.masks import make_identity

        make_identity(nc, ident_f32[:])
        probs = const_pool.tile([128, BC * SPS], f32, tag="probs")

        xbf_by_b = [None] * BC
        cast_insts_by_b = [None] * BC

        praw_by_b = [None] * BC

        p16_by_bc = {}

        def pool_front(b, ch):
            """Refold the chunk's probs into a [1, 512] bf16 row via a PE
            transpose + DRAM bounce (probs live in [128t, slice] layout)."""
            gs = b * SPS + ch * JJ
            pcols = probs[:, gs : gs + JJ]
            ptp = tiny_ps.tile([JJ, 128], f32, tag="tiny", name=f"ptp{b}_{ch}")
            nc.tensor.transpose(ptp[:], pcols, ident_f32[:])
            prow = sm_pool.tile([JJ, 128], f32, tag="prow")
            nc.vector.tensor_copy(prow[:], ptp[:])
            pscr = dram_pool.tile([1, CHUNK], f32, tag="pscr", name=f"pscr{b}_{ch}")
            wi = nc.scalar.dma_start(
                out=pscr[:].rearrange("o (j p) -> (o j) p", p=128), in_=prow[:]
            )
            p16 = sm_pool.tile([1, CHUNK], bf16, tag="p16")
            ri = nc.gpsimd.dma_start(out=p16[:], in_=pscr[:])
            tile.add_dep_helper(ri.ins, wi.ins, reason="pscr RAW")
            p16_by_bc[(b, ch)] = p16

        def pool_back(b, ch, xts):
            """Unnormalized pooling for chunk ch of sample b on the DVE:
            praw[:, hb, ch] = sum_{t in chunk} probs[t] * XT[hb][:, t]."""
            if praw_by_b[b] is None:
                praw_by_b[b] = sm_pool.tile(
                    [128, HB, CH_PER_B], f32, tag="praw", name=f"praw{b}"
                )
            praw = praw_by_b[b]
            p16 = p16_by_bc.pop((b, ch))
            pbp = abc_ps.tile([128, CHUNK], f32, tag="abc")
            nc.tensor.matmul(pbp[:], ones_row_b[:], p16[:], start=True, stop=True)
            pb_sb = sm_pool.tile([128, CHUNK], bf16, tag="pb_sb")
            nc.vector.tensor_copy(pb_sb[:], pbp[:])
            for hb in range(HB):
                junk = junk_pool.tile([128, CHUNK], bf16, tag="junk")
                nc.vector.tensor_mul(
                    junk[:], xts[hb][:, ch * CHUNK : (ch + 1) * CHUNK], pb_sb[:]
                )
                nc.vector.reduce_sum(praw[:, hb, ch : ch + 1], junk[:], axis=AX.X)

        def sample_tail(b):
            """softmax normalization + outputs for sample b."""
            pcols = probs[:, b * SPS : (b + 1) * SPS]
            partial = sm_pool.tile([128, 1], f32, tag="partial")
            nc.vector.reduce_sum(partial[:], pcols, axis=AX.X)
            tps = tiny_ps.tile([1, 1], f32, tag="tiny")
            nc.tensor.matmul(tps[:], ones_col[:], partial[:], start=True, stop=True)
            tsb = sm_pool.tile([1, 1], f32, tag="tsb")
            nc.vector.tensor_copy(tsb[:], tps[:])
            bps = tiny_ps.tile([128, 1], f32, tag="tiny")
            nc.tensor.matmul(bps[:], ones_row[:], tsb[:], start=True, stop=True)
            rsb = sm_pool.tile([128, 1], f32, tag="rsb")
            nc.vector.reciprocal(rsb[:], bps[:])
            attn_f = sm_pool.tile([128, SPS], f32, tag="attn_f")
            nc.vector.tensor_scalar_mul(attn_f[:], pcols, rsb[:])
            # transpose attn to [16 slices, 128 t] rows: contiguous output DMA
            atp = tiny_ps.tile([SPS, 128], f32, tag="tiny", name=f"atp{b}")
            nc.tensor.transpose(atp[:], attn_f[:], ident_f32[:])
            arow = sm_pool.tile([SPS, 128], f32, tag="arow")
            nc.vector.tensor_copy(arow[:], atp[:])
            nc.scalar.dma_start(
                out=out_attn[b : b + 1, :].rearrange("o (s p) -> (o s) p", p=128),
                in_=arow[:],
            )
            # ctx = praw summed over chunks, normalized by 1/sum(exp)
            ctxraw = sm_pool.tile([128, HB], f32, tag="ctxraw")
            nc.vector.reduce_sum(ctxraw[:], praw_by_b[b][:], axis=AX.X)
            ctxT = sm_pool.tile([128, HB], f32, tag="ctxT")
            nc.vector.tensor_scalar_mul(ctxT[:], ctxraw[:], rsb[:])
            nc.scalar.dma_start(
                out=out_ctx[b : b + 1, :].rearrange("o (hb p) -> p (o hb)", p=128),
                in_=ctxT[:],
            )

        def main_mms(b, ch, xts):
            """main matmuls + tanh for chunk ch of sample b; returns st tiles."""
            sts = []
            for hob in range(HB):
                ps = mm_ps.tile([128, CHUNK], f32, tag="mm")
                for hib in range(HB):
                    nc.tensor.matmul(
                        ps[:],
                        w_sb[:, hib, hob * 128 : (hob + 1) * 128],
                        xts[hib][:, ch * CHUNK : (ch + 1) * CHUNK],
                        start=(hib == 0),
                        stop=(hib == HB - 1),
                    )
                st = st_pool.tile([128, CHUNK], bf16, tag="st")
                nc.scalar.activation(st[:], ps[:], AF.Tanh)
                sts.append(st)
            return sts

        def logits_part(b, ch, sts):
            """logits^T matmuls + exp for chunk ch of sample b."""
            lg = lg_ps.tile([128, JJ], f32, tag="lg")
            for j in range(JJ):
                for hob in range(HB):
                    nc.tensor.matmul(
                        lg[:, j : j + 1],
                        sts[hob][:, j * 128 : (j + 1) * 128],
                        c_sb[:, hob : hob + 1],
                        start=(hob == 0),
                        stop=(hob == HB - 1),
                    )
            gs = b * SPS + ch * JJ  # global slice index
            nc.scalar.activation(probs[:, gs : gs + JJ], lg[:], AF.Exp)

        def emit_casts(b):
            """P0: cast sample b's x f32 -> bf16 natural-layout DRAM scratch.
            Contiguous reads/writes — one cast per 512-row chunk."""
            xbf = dram_pool.tile([T, H], bf16, tag="xbf", name=f"xbf{b}")
            xbf_by_b[b] = xbf
            cast_insts = []
            for ch in range(CH_PER_B):
                ci = nc.gpsimd.dma_start(
                    out=xbf[ch * CHUNK : (ch + 1) * CHUNK, :],
                    in_=xf[b * T + ch * CHUNK : b * T + (ch + 1) * CHUNK, :],
                )
                cast_insts.append(ci)
            cast_insts_by_b[b] = cast_insts

        def emit_transposes(b, c0, c1, xts=None):
            # XT: xbar transposes, all on the SP ring — two concurrent xbar
            # streams (sync + scalar) corrupt data. The strided-row source
            # (256B rows, 2KB stride) is the xbar's supported mid-dim case.
            if xts is None:
                xts = [
                    xt_pool.tile([128, T], bf16, tag="xt", name=f"xt{b}_{hb}")
                    for hb in range(HB)
                ]
            for hb in range(HB):
                ti = nc.sync.dma_start(
                    out=xts[hb][:, c0 * CHUNK : c1 * CHUNK],
                    in_=xbf_by_b[b][c0 * CHUNK : c1 * CHUNK, hb * 128 : (hb + 1) * 128],
                    transpose=True,
                )
                for ci in cast_insts_by_b[b][c0:c1]:
                    tile.add_dep_helper(ti.ins, ci.ins, reason="xnat RAW")
            return xts

        # Software-pipelined emission: casts one sample ahead; transposes
        # half-a-sample ahead (per-chunk for sample 0's startup); logits one
        # chunk behind main MMs; pooling two chunks behind; tails later still.
        emit_casts(0)
        xts_by_b = {}
        for ch in range(CH_PER_B):
            xts_by_b[0] = emit_transposes(0, ch, ch + 1, xts_by_b.get(0))
        pend_lg = None     # (b, ch, sts) awaiting logits emission (lag 1)
        pend_front = None  # (b, ch) awaiting pool_front emission (lag 2)
        pend_back = None   # (b, ch) awaiting pool_back emission (lag 3)
        pend_tail = None   # sample awaiting softmax normalization + outputs
        for b in range(BC):
            if b + 1 < BC:
                emit_casts(b + 1)
                xts_by_b[b + 1] = emit_transposes(b + 1, 0, 2)
            for ch in range(CH_PER_B):
                if ch == 2 and b + 1 < BC:
                    emit_transposes(b + 1, 2, 4, xts_by_b[b + 1])
                sts = main_mms(b, ch, xts_by_b[b])
                if pend_lg is not None:
                    logits_part(*pend_lg)
                if pend_front is not None:
                    pool_front(*pend_front)
                if pend_back is not None:
                    pool_back(*pend_back, xts_by_b[pend_back[0]])
                pend_back = pend_front
                pend_front = (pend_lg[0], pend_lg[1]) if pend_lg else None
                pend_lg = (b, ch, sts)
                if ch == 3 and pend_tail is not None:
                    sample_tail(pend_tail)
                    pend_tail = None
            xts_by_b.pop(b - 1, None)
            pend_tail = b
        logits_part(*pend_lg)
        pool_front(*pend_front)
        pool_back(*pend_back, xts_by_b[pend_back[0]])
        pool_front(pend_lg[0], pend_lg[1])
        pool_back(pend_front[0], pend_front[1], xts_by_b[pend_front[0]])
        pool_back(pend_lg[0], pend_lg[1], xts_by_b[pend_lg[0]])
        sample_tail(pend_tail)

    nc.compile()
    return nc


def _get_nc():
    if "nc" not in _cache:
        _cache["nc"] = build()
    return _cache["nc"]


def kernel(gru_output, attention_weights, context_vector):
    from concourse.bass_utils import run_bass_kernel_spmd

    nc = _get_nc()
    in_maps = []
    for i in range(NCORES):
        in_maps.append(
            {
                "x": np.ascontiguousarray(
                    gru_output[i * BC : (i + 1) * BC], dtype=np.float32
                ),
                "w": np.ascontiguousarray(attention_weights, dtype=np.float32),
                "c": np.ascontiguousarray(context_vector, dtype=np.float32),
            }
        )
    res = run_bass_kernel_spmd(nc, in_maps, list(range(NCORES))).results
    context = np.concatenate([res[i]["out_ctx"] for i in range(NCORES)], axis=0)
    attn = np.concatenate([res[i]["out_attn"] for i in range(NCORES)], axis=0)
    return context, attn


# revision 25
# speedup vs baseline: 1.1067x; 1.1067x over previous
"""Trainium2 Bass kernel for attention-pooling:
    score  = tanh(X @ W)            [B,T,H]
    logits = score @ c              [B,T]
    attn   = softmax(logits, ax=1)  [B,T]
    ctx    = attn-weighted sum of X over T -> [B,H]
Returns (ctx, attn). Data-parallel over batch across 8 NeuronCores.

Per-core pipeline (4 samples, T=2048, H=1024):
  P0   gpsimd cast-DMA x f32 -> bf16 natural DRAM scratch (contiguous),
       chained depth-2 so early chunks aren't starved by later casts
  XT   xbar DMA-transposes [tchunk,128]->[128,tchunk] bf16, single ring
       (two concurrent xbar streams corrupt data)
  A    S^T = W^T X^T in bf16 (W stationary), tanh fused on ScalarE
  L    logits^T via N=1 matmuls -> [128t, 1] slices (softmax-friendly layout)
  SM   exp (no max-sub: |logits| <= sum|c| ~ 40), cross-partition sum via
       ones-matmul, reciprocal broadcast, attn = probs * recip
  P    pooling on DVE: ctx^T[h] = sum_t XT[h,t]*attn_bcast[h,t], with attn
       broadcast across partitions via K=1 matmuls
"""

import numpy as np

B, T, H = 32, 2048, 1024
NCORES = 8
BC = B // NCORES            # samples per core
CHUNK = 512                 # t-rows per phase-A chunk
CH_PER_B = T // CHUNK       # 4 chunks per sample
HB = H // 128               # 8 h blocks
JJ = CHUNK // 128            # 4 128-t slices per chunk
SPS = T // 128              # 16 slices per sample

_cache = {}


def build():
    import concourse.bass as bass
    import concourse.tile as tile
    from concourse import bacc, mybir
    from concourse.masks import make_identity
    from contextlib import ExitStack

    f32 = mybir.dt.float32
    bf16 = mybir.dt.bfloat16
    AF = mybir.ActivationFunctionType
    AX = mybir.AxisListType

    nc = bacc.Bacc("TRN2", target_bir_lowering=False, debug=False)

    x = nc.declare_dram_parameter("x", [BC, T, H], f32, isOutput=False)
    w = nc.declare_dram_parameter("w", [H, H], f32, isOutput=False)
    c = nc.declare_dram_parameter("c", [H, 1], f32, isOutput=False)
    out_ctx = nc.declare_dram_parameter("out_ctx", [BC, H], f32, isOutput=True)
    out_attn = nc.declare_dram_parameter("out_attn", [BC, T], f32, isOutput=True)

    xf = x[:].rearrange("b t h -> (b t) h")  # [8192, 1024]

    with tile.TileContext(nc) as tc, ExitStack() as ctx:
        ep = ctx.enter_context
        const_pool = ep(tc.tile_pool(name="const", bufs=1))
        xt_pool = ep(tc.tile_pool(name="xt", bufs=24))
        st_pool = ep(tc.tile_pool(name="st", bufs=16))
        junk_pool = ep(tc.tile_pool(name="junk", bufs=2))
        sm_pool = ep(tc.tile_pool(name="sm", bufs=2))
        dram_pool = ep(tc.tile_pool(name="dram", bufs=4, space="DRAM"))
        mm_ps = ep(tc.tile_pool(name="mm_ps", bufs=3, space="PSUM"))
        lg_ps = ep(tc.tile_pool(name="lg_ps", bufs=1, space="PSUM"))
        abc_ps = ep(tc.tile_pool(name="abc_ps", bufs=2, space="PSUM"))
        tiny_ps = ep(tc.tile_pool(name="tiny_ps", bufs=2, space="PSUM"))

        # ---- constants / weights (issued before the x casts: W gates MM 0) --
        w_sb = const_pool.tile([128, HB, H], bf16, tag="w")
        nc.gpsimd.dma_start(
            out=w_sb[:], in_=w[:].rearrange("(hb p) h -> p hb h", p=128)
        )
        c_sb = const_pool.tile([128, HB], bf16, tag="c")
        nc.gpsimd.dma_start(
            out=c_sb[:], in_=c[:].rearrange("(a p) k -> p (a k)", p=128)
        )
        ones_col = const_pool.tile([128, 1], f32, tag="ones_col")
        nc.any.memset(ones_col[:], 1.0)
        ones_row = const_pool.tile([1, 128], f32, tag="ones_row")
        nc.any.memset(ones_row[:], 1.0)
        ones_row_b = const_pool.tile([1, 128], bf16, tag="ones_row_b")
        nc.any.memset(ones_row_b[:], 1.0)
        ident_f32 = const_pool.tile([128, 128], f32, tag="ident")
        make_identity(nc, ident_f32[:])
        probs = const_pool.tile([128, BC * SPS], f32, tag="probs")

        xbf_by_b = [None] * BC
        cast_insts_by_b = [None] * BC
        all_cast_insts = []

        def sample_tail(b, xts):
            """softmax + DVE pooling for sample b (all its logits in probs)."""
            pcols = probs[:, b * SPS : (b + 1) * SPS]
            partial = sm_pool.tile([128, 1], f32, tag="partial")
            nc.vector.reduce_sum(partial[:], pcols, axis=AX.X)
            tps = tiny_ps.tile([1, 1], f32, tag="tiny")
            nc.tensor.matmul(tps[:], ones_col[:], partial[:], start=True, stop=True)
            tsb = sm_pool.tile([1, 1], f32, tag="tsb")
            nc.vector.tensor_copy(tsb[:], tps[:])
            bps = tiny_ps.tile([128, 1], f32, tag="tiny")
            nc.tensor.matmul(bps[:], ones_row[:], tsb[:], start=True, stop=True)
            rsb = sm_pool.tile([128, 1], f32, tag="rsb")
            nc.vector.reciprocal(rsb[:], bps[:])
            attn_f = sm_pool.tile([128, SPS], f32, tag="attn_f")
            nc.vector.tensor_scalar_mul(attn_f[:], pcols, rsb[:])
            # transpose attn to [16 slices, 128 t] rows: contiguous output DMA
            atp = tiny_ps.tile([SPS, 128], f32, tag="tiny", name=f"atp{b}")
            nc.tensor.transpose(atp[:], attn_f[:], ident_f32[:])
            arow = sm_pool.tile([SPS, 128], f32, tag="arow")
            nc.vector.tensor_copy(arow[:], atp[:])
            nc.scalar.dma_start(
                out=out_attn[b : b + 1, :].rearrange("o (s p) -> (o s) p", p=128),
                in_=arow[:],
            )
            # bounce attn through DRAM scratch to refold [16,128] -> [1,2048]
            ascr = dram_pool.tile([1, T], f32, tag="ascr", name=f"ascr{b}")
            wi = nc.scalar.dma_start(
                out=ascr[:].rearrange("o (s p) -> (o s) p", p=128), in_=arow[:]
            )
            a16 = sm_pool.tile([1, T], bf16, tag="a16")
            ri = nc.gpsimd.dma_start(out=a16[:], in_=ascr[:])
            tile.add_dep_helper(ri.ins, wi.ins, reason="ascr RAW")
            # broadcast attn row across partitions: ab[p, t] = attn[t]
            ab = sm_pool.tile([128, T], bf16, tag="ab")
            for q in range(CH_PER_B):
                abp = abc_ps.tile([128, 512], f32, tag="abc")
                nc.tensor.matmul(
                    abp[:],
                    ones_row_b[:],
                    a16[0:1, q * 512 : (q + 1) * 512],
                    start=True,
                    stop=True,
                )
                nc.vector.tensor_copy(ab[:, q * 512 : (q + 1) * 512], abp[:])
            # pooling on DVE: ctxT[h, hb] = sum_t XT[hb][h, t] * attn[t]
            ctxT = sm_pool.tile([128, HB], f32, tag="ctxT")
            for hb in range(HB):
                junk = junk_pool.tile([128, T], bf16, tag="junk")
                nc.vector.tensor_mul(junk[:], xts[hb][:], ab[:])
                nc.vector.reduce_sum(ctxT[:, hb : hb + 1], junk[:], axis=AX.X)
            nc.scalar.dma_start(
                out=out_ctx[b : b + 1, :].rearrange("o (hb p) -> p (o hb)", p=128),
                in_=ctxT[:],
            )

        def main_mms(b, ch, xts):
            """main matmuls + tanh for chunk ch of sample b; returns st tiles."""
            sts = []
            for hob in range(HB):
                ps = mm_ps.tile([128, CHUNK], f32, tag="mm")
                for hib in range(HB):
                    nc.tensor.matmul(
                        ps[:],
                        w_sb[:, hib, hob * 128 : (hob + 1) * 128],
                        xts[hib][:, ch * CHUNK : (ch + 1) * CHUNK],
                        start=(hib == 0),
                        stop=(hib == HB - 1),
                    )
                st = st_pool.tile([128, CHUNK], bf16, tag="st")
                nc.scalar.activation(st[:], ps[:], AF.Tanh)
                sts.append(st)
            return sts

        def logits_part(b, ch, sts):
            """logits^T matmuls + exp for chunk ch of sample b."""
            lg = lg_ps.tile([128, JJ], f32, tag="lg")
            for j in range(JJ):
                for hob in range(HB):
                    nc.tensor.matmul(
                        lg[:, j : j + 1],
                        sts[hob][:, j * 128 : (j + 1) * 128],
                        c_sb[:, hob : hob + 1],
                        start=(hob == 0),
                        stop=(hob == HB - 1),
                    )
            gs = b * SPS + ch * JJ  # global slice index
            nc.scalar.activation(probs[:, gs : gs + JJ], lg[:], AF.Exp)

        def emit_casts(b):
            """P0: cast sample b's x f32 -> bf16 natural-layout DRAM scratch.
            Each cast waits on the cast two back, bounding how many bulk DMAs
            compete on the SDMA engines (they round-robin at packet level, so
            unbounded concurrency makes ALL chunks finish late together)."""
            xbf = dram_pool.tile([T, H], bf16, tag="xbf", name=f"xbf{b}")
            xbf_by_b[b] = xbf
            cast_insts = []
            for ch in range(CH_PER_B):
                ci = nc.gpsimd.dma_start(
                    out=xbf[ch * CHUNK : (ch + 1) * CHUNK, :],
                    in_=xf[b * T + ch * CHUNK : b * T + (ch + 1) * CHUNK, :],
                )
                if len(all_cast_insts) >= 2:
                    tile.add_dep_helper(
                        ci.ins, all_cast_insts[-2].ins, reason="cast chain"
                    )
                cast_insts.append(ci)
                all_cast_insts.append(ci)
            cast_insts_by_b[b] = cast_insts

        def emit_transposes(b, c0, c1, xts=None):
            # XT: xbar transposes, all on the SP ring — two concurrent xbar
            # streams (sync + scalar) corrupt data. The strided-row source
            # (256B rows, 2KB stride) is the xbar's supported mid-dim case.
            if xts is None:
                xts = [
                    xt_pool.tile([128, T], bf16, tag="xt", name=f"xt{b}_{hb}")
                    for hb in range(HB)
                ]
            for hb in range(HB):
                ti = nc.sync.dma_start(
                    out=xts[hb][:, c0 * CHUNK : c1 * CHUNK],
                    in_=xbf_by_b[b][c0 * CHUNK : c1 * CHUNK, hb * 128 : (hb + 1) * 128],
                    transpose=True,
                )
                for ci in cast_insts_by_b[b][c0:c1]:
                    tile.add_dep_helper(ti.ins, ci.ins, reason="xnat RAW")
            return xts

        # Software-pipelined emission: casts one sample ahead (chained);
        # transposes per-chunk for sample 0's startup, per-half after; logits
        # one chunk behind main MMs; sample tails in the next sample's chunk 1.
        emit_casts(0)
        xts_by_b = {}
        for ch in range(CH_PER_B):
            xts_by_b[0] = emit_transposes(0, ch, ch + 1, xts_by_b.get(0))
        pend_lg = None   # (b, ch, sts) awaiting logits emission
        pend_tail = None  # sample awaiting softmax+pooling emission
        for b in range(BC):
            if b + 1 < BC:
                emit_casts(b + 1)
                xts_by_b[b + 1] = emit_transposes(b + 1, 0, 2)
            for ch in range(CH_PER_B):
                if ch == 2 and b + 1 < BC:
                    emit_transposes(b + 1, 2, 4, xts_by_b[b + 1])
                sts = main_mms(b, ch, xts_by_b[b])
                if pend_lg is not None:
                    logits_part(*pend_lg)
                pend_lg = (b, ch, sts)
                if ch == 1 and pend_tail is not None:
                    sample_tail(pend_tail, xts_by_b[pend_tail])
                    pend_tail = None
            xts_by_b.pop(b - 1, None)
            pend_tail = b
        logits_part(*pend_lg)
        sample_tail(pend_tail, xts_by_b[pend_tail])

    nc.compile()
    return nc


def _get_nc():
    if "nc" not in _cache:
        _cache["nc"] = build()
    return _cache["nc"]


def kernel(gru_output, attention_weights, context_vector):
    from concourse.bass_utils import run_bass_kernel_spmd

    nc = _get_nc()
    in_maps = []
    for i in range(NCORES):
        in_maps.append(
            {
                "x": np.ascontiguousarray(
                    gru_output[i * BC : (i + 1) * BC], dtype=np.float32
                ),
                "w": np.ascontiguousarray(attention_weights, dtype=np.float32),
                "c": np.ascontiguousarray(context_vector, dtype=np.float32),
            }
        )
    res = run_bass_kernel_spmd(nc, in_maps, list(range(NCORES))).results
    context = np.concatenate([res[i]["out_ctx"] for i in range(NCORES)], axis=0)
    attn = np.concatenate([res[i]["out_attn"] for i in range(NCORES)], axis=0)
    return context, attn


# revision 27
# speedup vs baseline: 1.1711x; 1.0582x over previous
"""Trainium2 Bass kernel for attention-pooling:
    score  = tanh(X @ W)            [B,T,H]
    logits = score @ c              [B,T]
    attn   = softmax(logits, ax=1)  [B,T]
    ctx    = attn-weighted sum of X over T -> [B,H]
Returns (ctx, attn). Data-parallel over batch across 8 NeuronCores.

Per-core pipeline (4 samples, T=2048, H=1024):
  P0   gpsimd cast-DMA x f32 -> bf16 natural DRAM scratch (contiguous),
       chained depth-2 so early chunks aren't starved by later casts
  XT   xbar DMA-transposes [tchunk,128]->[128,tchunk] bf16, single ring
       (two concurrent xbar streams corrupt data)
  A    S^T = W^T X^T in bf16 (W stationary), tanh fused on ScalarE
  L    logits^T via N=1 matmuls -> [128t, 1] slices (softmax-friendly layout)
  SM   exp (no max-sub: |logits| <= sum|c| ~ 40), cross-partition sum via
       ones-matmul, reciprocal broadcast, attn = probs * recip
  P    pooling on DVE: ctx^T[h] = sum_t XT[h,t]*attn_bcast[h,t], with attn
       broadcast across partitions via K=1 matmuls
"""

import numpy as np

B, T, H = 32, 2048, 1024
NCORES = 8
BC = B // NCORES            # samples per core
CHUNK = 512                 # t-rows per phase-A chunk
CH_PER_B = T // CHUNK       # 4 chunks per sample
HB = H // 128               # 8 h blocks
JJ = CHUNK // 128            # 4 128-t slices per chunk
SPS = T // 128              # 16 slices per sample

_cache = {}


def build():
    import concourse.bass as bass
    import concourse.tile as tile
    from concourse import bacc, mybir
    from concourse.masks import make_identity
    from contextlib import ExitStack

    f32 = mybir.dt.float32
    bf16 = mybir.dt.bfloat16
    AF = mybir.ActivationFunctionType
    AX = mybir.AxisListType

    nc = bacc.Bacc("TRN2", target_bir_lowering=False, debug=False)

    x = nc.declare_dram_parameter("x", [BC, T, H], f32, isOutput=False)
    w = nc.declare_dram_parameter("w", [H, H], f32, isOutput=False)
    c = nc.declare_dram_parameter("c", [H, 1], f32, isOutput=False)
    out_ctx = nc.declare_dram_parameter("out_ctx", [BC, H], f32, isOutput=True)
    out_attn = nc.declare_dram_parameter("out_attn", [BC, T], f32, isOutput=True)

    xf = x[:].rearrange("b t h -> (b t) h")  # [8192, 1024]

    with tile.TileContext(nc) as tc, ExitStack() as ctx:
        ep = ctx.enter_context
        const_pool = ep(tc.tile_pool(name="const", bufs=1))
        xt_pool = ep(tc.tile_pool(name="xt", bufs=24))
        st_pool = ep(tc.tile_pool(name="st", bufs=16))
        junk_pool = ep(tc.tile_pool(name="junk", bufs=2))
        sm_pool = ep(tc.tile_pool(name="sm", bufs=2))
        dram_pool = ep(tc.tile_pool(name="dram", bufs=4, space="DRAM"))
        mm_ps = ep(tc.tile_pool(name="mm_ps", bufs=3, space="PSUM"))
        lg_ps = ep(tc.tile_pool(name="lg_ps", bufs=1, space="PSUM"))
        abc_ps = ep(tc.tile_pool(name="abc_ps", bufs=2, space="PSUM"))
        tiny_ps = ep(tc.tile_pool(name="tiny_ps", bufs=2, space="PSUM"))

        # ---- constants / weights (issued before the x casts: W gates MM 0) --
        w_sb = const_pool.tile([128, HB, H], bf16, tag="w")
        nc.gpsimd.dma_start(
            out=w_sb[:], in_=w[:].rearrange("(hb p) h -> p hb h", p=128)
        )
        c_sb = const_pool.tile([128, HB], bf16, tag="c")
        nc.gpsimd.dma_start(
            out=c_sb[:], in_=c[:].rearrange("(a p) k -> p (a k)", p=128)
        )
        ones_col = const_pool.tile([128, 1], f32, tag="ones_col")
        nc.any.memset(ones_col[:], 1.0)
        ones_row = const_pool.tile([1, 128], f32, tag="ones_row")
        nc.any.memset(ones_row[:], 1.0)
        ones_row_b = const_pool.tile([1, 128], bf16, tag="ones_row_b")
        nc.any.memset(ones_row_b[:], 1.0)
        ident_f32 = const_pool.tile([128, 128], f32, tag="ident")
        make_identity(nc, ident_f32[:])
        probs = const_pool.tile([128, BC * SPS], f32, tag="probs")

        xbf_by_b = [None] * BC
        cast_insts_by_b = [None] * BC
        all_cast_insts = []

        def sample_tail(b, xts):
            """softmax + DVE pooling for sample b (all its logits in probs)."""
            pcols = probs[:, b * SPS : (b + 1) * SPS]
            partial = sm_pool.tile([128, 1], f32, tag="partial")
            nc.vector.reduce_sum(partial[:], pcols, axis=AX.X)
            tps = tiny_ps.tile([1, 1], f32, tag="tiny")
            nc.tensor.matmul(tps[:], ones_col[:], partial[:], start=True, stop=True)
            tsb = sm_pool.tile([1, 1], f32, tag="tsb")
            nc.vector.tensor_copy(tsb[:], tps[:])
            bps = tiny_ps.tile([128, 1], f32, tag="tiny")
            nc.tensor.matmul(bps[:], ones_row[:], tsb[:], start=True, stop=True)
            rsb = sm_pool.tile([128, 1], f32, tag="rsb")
            nc.vector.reciprocal(rsb[:], bps[:])
            attn_f = sm_pool.tile([128, SPS], f32, tag="attn_f")
            nc.vector.tensor_scalar_mul(attn_f[:], pcols, rsb[:])
            # transpose attn to [16 slices, 128 t] rows: contiguous output DMA
            atp = tiny_ps.tile([SPS, 128], f32, tag="tiny", name=f"atp{b}")
            nc.tensor.transpose(atp[:], attn_f[:], ident_f32[:])
            arow = sm_pool.tile([SPS, 128], f32, tag="arow")
            nc.vector.tensor_copy(arow[:], atp[:])
            nc.scalar.dma_start(
                out=out_attn[b : b + 1, :].rearrange("o (s p) -> (o s) p", p=128),
                in_=arow[:],
            )
            # bounce attn through DRAM scratch to refold [16,128] -> [1,2048]
            ascr = dram_pool.tile([1, T], f32, tag="ascr", name=f"ascr{b}")
            wi = nc.scalar.dma_start(
                out=ascr[:].rearrange("o (s p) -> (o s) p", p=128), in_=arow[:]
            )
            a16 = sm_pool.tile([1, T], bf16, tag="a16")
            ri = nc.gpsimd.dma_start(out=a16[:], in_=ascr[:])
            tile.add_dep_helper(ri.ins, wi.ins, reason="ascr RAW")
            # broadcast attn row across partitions: ab[p, t] = attn[t]
            ab = sm_pool.tile([128, T], bf16, tag="ab")
            for q in range(CH_PER_B):
                abp = abc_ps.tile([128, 512], f32, tag="abc")
                nc.tensor.matmul(
                    abp[:],
                    ones_row_b[:],
                    a16[0:1, q * 512 : (q + 1) * 512],
                    start=True,
                    stop=True,
                )
                nc.vector.tensor_copy(ab[:, q * 512 : (q + 1) * 512], abp[:])
            # pooling on DVE: ctxT[h, hb] = sum_t XT[hb][h, t] * attn[t]
            ctxT = sm_pool.tile([128, HB], f32, tag="ctxT")
            for hb in range(HB):
                junk = junk_pool.tile([128, T], bf16, tag="junk")
                nc.vector.tensor_mul(junk[:], xts[hb][:], ab[:])
                nc.vector.reduce_sum(ctxT[:, hb : hb + 1], junk[:], axis=AX.X)
            nc.scalar.dma_start(
                out=out_ctx[b : b + 1, :].rearrange("o (hb p) -> p (o hb)", p=128),
                in_=ctxT[:],
            )

        def main_mms(b, ch, xts):
            """main matmuls + tanh for chunk ch of sample b; returns st tiles."""
            sts = []
            for hob in range(HB):
                ps = mm_ps.tile([128, CHUNK], f32, tag="mm")
                for hib in range(HB):
                    nc.tensor.matmul(
                        ps[:],
                        w_sb[:, hib, hob * 128 : (hob + 1) * 128],
                        xts[hib][:, ch * CHUNK : (ch + 1) * CHUNK],
                        start=(hib == 0),
                        stop=(hib == HB - 1),
                    )
                st = st_pool.tile([128, CHUNK], bf16, tag="st")
                nc.scalar.activation(st[:], ps[:], AF.Tanh)
                sts.append(st)
            return sts

        def logits_part(b, ch, sts):
            """logits^T matmuls + exp for chunk ch of sample b."""
            lg = lg_ps.tile([128, JJ], f32, tag="lg")
            for j in range(JJ):
                for hob in range(HB):
                    nc.tensor.matmul(
                        lg[:, j : j + 1],
                        sts[hob][:, j * 128 : (j + 1) * 128],
                        c_sb[:, hob : hob + 1],
                        start=(hob == 0),
                        stop=(hob == HB - 1),
                    )
            gs = b * SPS + ch * JJ  # global slice index
            nc.scalar.activation(probs[:, gs : gs + JJ], lg[:], AF.Exp)

        def emit_casts(b):
            """P0: cast sample b's x f32 -> bf16 natural-layout DRAM scratch.
            Each cast waits on the cast two back, bounding how many bulk DMAs
            compete on the SDMA engines (they round-robin at packet level, so
            unbounded concurrency makes ALL chunks finish late together)."""
            xbf = dram_pool.tile([T, H], bf16, tag="xbf", name=f"xbf{b}")
            xbf_by_b[b] = xbf
            cast_insts = []
            for ch in range(CH_PER_B):
                ci = nc.gpsimd.dma_start(
                    out=xbf[ch * CHUNK : (ch + 1) * CHUNK, :],
                    in_=xf[b * T + ch * CHUNK : b * T + (ch + 1) * CHUNK, :],
                )
                cast_insts.append(ci)
                all_cast_insts.append(ci)
            cast_insts_by_b[b] = cast_insts

        def emit_transposes(b, c0, c1, xts=None):
            # XT: xbar transposes, all on the SP ring — two concurrent xbar
            # streams (sync + scalar) corrupt data. The strided-row source
            # (256B rows, 2KB stride) is the xbar's supported mid-dim case.
            if xts is None:
                xts = [
                    xt_pool.tile([128, T], bf16, tag="xt", name=f"xt{b}_{hb}")
                    for hb in range(HB)
                ]
            for hb in range(HB):
                ti = nc.sync.dma_start(
                    out=xts[hb][:, c0 * CHUNK : c1 * CHUNK],
                    in_=xbf_by_b[b][c0 * CHUNK : c1 * CHUNK, hb * 128 : (hb + 1) * 128],
                    transpose=True,
                )
                for ci in cast_insts_by_b[b][c0:c1]:
                    tile.add_dep_helper(ti.ins, ci.ins, reason="xnat RAW")
            return xts

        # Software-pipelined emission: casts one sample ahead (chained);
        # transposes per-chunk for sample 0's startup, per-half after; logits
        # one chunk behind main MMs; sample tails in the next sample's chunk 1.
        emit_casts(0)
        xts_by_b = {0: emit_transposes(0, 0, 2)}
        emit_transposes(0, 2, 4, xts_by_b[0])
        pend_lg = None   # (b, ch, sts) awaiting logits emission
        pend_tail = None  # sample awaiting softmax+pooling emission
        for b in range(BC):
            if b + 1 < BC:
                emit_casts(b + 1)
                xts_by_b[b + 1] = emit_transposes(b + 1, 0, 2)
            for ch in range(CH_PER_B):
                if ch == 2 and b + 1 < BC:
                    emit_transposes(b + 1, 2, 4, xts_by_b[b + 1])
                sts = main_mms(b, ch, xts_by_b[b])
                if pend_lg is not None:
                    logits_part(*pend_lg)
                pend_lg = (b, ch, sts)
                if ch == 1 and pend_tail is not None:
                    sample_tail(pend_tail, xts_by_b[pend_tail])
                    pend_tail = None
            xts_by_b.pop(b - 1, None)
            pend_tail = b
        logits_part(*pend_lg)
        sample_tail(pend_tail, xts_by_b[pend_tail])

    nc.compile()
    return nc


def _get_nc():
    if "nc" not in _cache:
        _cache["nc"] = build()
    return _cache["nc"]


def kernel(gru_output, attention_weights, context_vector):
    from concourse.bass_utils import run_bass_kernel_spmd

    nc = _get_nc()
    in_maps = []
    for i in range(NCORES):
        in_maps.append(
            {
                "x": np.ascontiguousarray(
                    gru_output[i * BC : (i + 1) * BC], dtype=np.float32
                ),
                "w": np.ascontiguousarray(attention_weights, dtype=np.float32),
                "c": np.ascontiguousarray(context_vector, dtype=np.float32),
            }
        )
    res = run_bass_kernel_spmd(nc, in_maps, list(range(NCORES))).results
    context = np.concatenate([res[i]["out_ctx"] for i in range(NCORES)], axis=0)
    attn = np.concatenate([res[i]["out_attn"] for i in range(NCORES)], axis=0)
    return context, attn
